# revision 3
# baseline (speedup 1.0000x reference)
"""Trainium2 Bass kernel for the BEMv13 MoE-LoRA module.

Computation (per token t, full problem):
  base  = x @ W_base.T + b_base
  w     = softmax(x @ W_router + b_router)        # E=2 experts
  H     = x @ A_cat.T                             # [T, 16] LoRA down-proj, both experts
  G     = H * w_broadcast * (alpha/rank)          # per-expert routing weight
  out   = base + G @ B_cat.T

Sharding: tokens (batch*seq = 16384) split evenly across 8 NeuronCores;
all weights replicated. No cross-core communication.

On-core algorithm (per core, 2048 tokens):
  - W_base is pre-transposed on host to W^T [D, O]; rounded on-chip to
    float32r (TF32-like) and kept resident in SBUF (128 KB/partition).
  - x arrives token-major; each [128,128] tile is transposed on the PE
    (fp32 transpose) and rounded to float32r during the PSUM->SBUF drain.
  - Main matmul: out[128 tok, 512 o] accumulated over 16 k-tiles in PSUM,
    float32r operands (1 cycle/row, ~fp32 dynamic range, ~1.3e-4 rel err).
  - Router logits difference and LoRA H are fused into one small rhs
    (aat, [D, 18]) sharing the same stationary x^T tiles.
  - softmax over 2 experts == sigmoid of the logit difference.
  - G^T (PE transpose of the scaled H) feeds a final K=16 accumulation
    step with B_cat^T, so the LoRA up-proj lands in the same PSUM banks.
"""

import numpy as np

P = 128
D = 2048
O = 2048
KT = D // P            # 16 k-tiles
TOK = 2048             # tokens per core
NSTR = TOK // 256      # 8 stripes of 256 tokens (2 tok-tiles)
HN = 18                # 16 LoRA cols + 1 router-diff col + 1 pad (fp32r needs even N)
ER = 16                # E*R
SCALE = 16.0 / 8.0
NCORES = 8

_CACHE = {}


def _build():
    import concourse.tile as tile
    import concourse.masks as masks
    from concourse import bacc, mybir

    f32 = mybir.dt.float32
    f32r = mybir.dt.float32r

    nc = bacc.Bacc("TRN2", target_bir_lowering=False, debug=False)

    xs_d = nc.dram_tensor("xs", [TOK, D], f32, kind="ExternalInput")
    wt_d = nc.dram_tensor("wt", [D, O], f32, kind="ExternalInput")
    aat_d = nc.dram_tensor("aat", [D, HN], f32, kind="ExternalInput")
    bt_d = nc.dram_tensor("bt", [ER, O], f32, kind="ExternalInput")
    bb_d = nc.dram_tensor("bb", [1, O], f32, kind="ExternalInput")
    brd_d = nc.dram_tensor("brd", [1, 1], f32, kind="ExternalInput")
    out_d = nc.dram_tensor("out", [TOK, O], f32, kind="ExternalOutput")

    with tile.TileContext(nc) as tc:
        with (
            tc.tile_pool(name="res", bufs=1) as res,
            tc.tile_pool(name="big2k", bufs=2) as big2k,
            tc.tile_pool(name="xpool", bufs=1) as xpool,
            tc.tile_pool(name="xtpool", bufs=18) as xtpool,
            tc.tile_pool(name="small", bufs=2) as small,
            tc.tile_pool(name="psA", bufs=1, space="PSUM") as psA,
            tc.tile_pool(name="psT", bufs=2, space="PSUM") as psT,
            tc.tile_pool(name="psH", bufs=2, space="PSUM") as psH,
        ):
            ident = res.tile([P, P], f32, tag="ident")
            masks.make_identity(nc, ident[:])
            identr = res.tile([P, P], f32r, tag="identr")
            nc.vector.tensor_copy(identr[:], ident[:])

            # --- constants: biases (replicated across partitions), A_aug^T, B_cat^T
            bb128 = res.tile([P, O], f32, tag="bb128")
            bb1 = big2k.tile([P, O], f32, tag="big2k", name="bb1_stage")
            nc.sync.dma_start(bb1[:1, :], bb_d[:])
            nc.gpsimd.partition_broadcast(bb128[:], bb1[:1, :])

            brd128 = res.tile([P, 1], f32, tag="brd128")
            brd1 = res.tile([1, 1], f32, tag="brd1")
            nc.sync.dma_start(brd1[:], brd_d[:])
            nc.gpsimd.partition_broadcast(brd128[:], brd1[:])

            aat32 = res.tile([P, KT * HN], f32, tag="aat32")
            nc.sync.dma_start(
                aat32[:].rearrange("p (k e) -> p k e", e=HN),
                aat_d[:].rearrange("(k p) e -> p k e", p=P),
            )
            aat_r = res.tile([P, KT * HN], f32r, tag="aat_r")
            nc.vector.tensor_copy(aat_r[:], aat32[:])

            bt_r = res.tile([ER, O], f32r, tag="bt_r")
            bt32 = big2k.tile([P, O], f32, tag="big2k", name="bt32_stage")
            nc.sync.dma_start(bt32[:ER, :], bt_d[:])
            nc.vector.tensor_copy(bt_r[:], bt32[:ER, :])

            # --- W^T resident in SBUF as float32r: [d-part, k-tile * O]
            wt_r = res.tile([P, KT * O], f32r, tag="wt_r")
            for k in range(KT):
                w32 = big2k.tile([P, O], f32, tag="big2k", name=f"w32_{k}")
                nc.sync.dma_start(w32[:], wt_d[k * P:(k + 1) * P, :])
                nc.vector.tensor_copy(wt_r[:, k * O:(k + 1) * O], w32[:])

            # --- main stripe loop: 256 tokens (2 tok-tiles) per stripe
            for s in range(NSTR):
                x32 = xpool.tile([P, 2 * D], f32, tag="x32", name=f"x32_{s}")
                nc.sync.dma_start(
                    x32[:].rearrange("p (i d) -> p i d", d=D),
                    xs_d[s * 256:(s + 1) * 256, :].rearrange("(i p) d -> p i d", p=P),
                )

                # transpose x tiles: [128 tok, 128 d] -> [128 d, 256 tok] per k
                xts = []
                for k in range(KT):
                    xt = xtpool.tile([P, 256], f32r, tag="xt", name=f"xt_{s}_{k}")
                    xts.append(xt)
                for k2 in range(0, KT, 2):
                    st = psT.tile([P, 512], f32, tag="tstage", name=f"ts_{s}_{k2}")
                    for q in range(2):
                        for i in range(2):
                            nc.tensor.transpose(
                                st[:, q * 256 + i * P:q * 256 + (i + 1) * P],
                                x32[:, i * D + (k2 + q) * P:i * D + (k2 + q + 1) * P],
                                ident[:],
                            )
                    nc.vector.tensor_copy(xts[k2][:], st[:, 0:256])
                    nc.vector.tensor_copy(xts[k2 + 1][:], st[:, 256:512])

                for i in range(2):
                    accs = [
                        psA.tile([P, 512], f32, tag=f"acc{j}", name=f"acc_{s}_{i}_{j}")
                        for j in range(4)
                    ]
                    h = psH.tile([P, HN], f32, tag="h", name=f"h_{s}_{i}")
                    for k in range(KT):
                        lhs = xts[k][:, i * P:(i + 1) * P]
                        nc.tensor.matmul(h[:], lhs, aat_r[:, k * HN:(k + 1) * HN],
                                         start=(k == 0), stop=(k == KT - 1))
                        for j in range(4):
                            nc.tensor.matmul(
                                accs[j][:], lhs,
                                wt_r[:, k * O + j * 512:k * O + (j + 1) * 512],
                                start=(k == 0), stop=False,
                            )

                    # routing: w1 = sigmoid(dlogit + brd); scaled by alpha/rank
                    srow = small.tile([P, 1], f32, tag="srow", name=f"srow_{s}_{i}")
                    nc.scalar.activation(srow[:], h[:, ER:ER + 1],
                                         mybir.ActivationFunctionType.Sigmoid,
                                         bias=brd128[:, 0:1], scale=1.0)
                    w1s = small.tile([P, 1], f32, tag="w1s", name=f"w1s_{s}_{i}")
                    nc.vector.tensor_scalar_mul(w1s[:], srow[:], SCALE)
                    w0s = small.tile([P, 1], f32, tag="w0s", name=f"w0s_{s}_{i}")
                    nc.vector.tensor_scalar(w0s[:], srow[:], -SCALE, SCALE,
                                            mybir.AluOpType.mult, mybir.AluOpType.add)
                    g = small.tile([P, ER], f32r, tag="g", name=f"g_{s}_{i}")
                    nc.vector.tensor_scalar_mul(g[:, 0:8], h[:, 0:8], w0s[:])
                    nc.vector.tensor_scalar_mul(g[:, 8:16], h[:, 8:16], w1s[:])

                    gst = psT.tile([ER, P], f32r, tag="tstage", name=f"gst_{s}_{i}")
                    nc.tensor.transpose(gst[:], g[:], identr[:])
                    gt = small.tile([ER, P], f32r, tag="gt", name=f"gt_{s}_{i}")
                    nc.vector.tensor_copy(gt[:], gst[:])

                    for j in range(4):
                        nc.tensor.matmul(accs[j][:], gt[:],
                                         bt_r[:, j * 512:(j + 1) * 512],
                                         start=False, stop=True)

                    outt = big2k.tile([P, O], f32, tag="big2k", name=f"out_{s}_{i}")
                    for j in range(4):
                        nc.vector.tensor_add(outt[:, j * 512:(j + 1) * 512],
                                             accs[j][:], bb128[:, j * 512:(j + 1) * 512])
                    nc.sync.dma_start(
                        out_d[s * 256 + i * P:s * 256 + (i + 1) * P, :], outt[:])

    nc.compile()
    return nc


def _prep_host(x, W_base, b_base, A, B, W_router, b_router):
    """Host-side layout prep + sharding. Returns per-core input maps."""
    x_flat = np.ascontiguousarray(x, dtype=np.float32).reshape(-1, D)
    wt = np.ascontiguousarray(W_base.T, dtype=np.float32)           # [D, O]
    a_cat = np.asarray(A, dtype=np.float32).reshape(ER, D)          # [16, D]
    aat = np.zeros((D, HN), dtype=np.float32)
    aat[:, :ER] = a_cat.T
    aat[:, ER] = np.asarray(W_router, dtype=np.float32)[:, 1] - np.asarray(W_router, dtype=np.float32)[:, 0]
    b_cat = np.concatenate([np.asarray(B, dtype=np.float32)[0],
                            np.asarray(B, dtype=np.float32)[1]], axis=1)  # [O, 16]
    bt = np.ascontiguousarray(b_cat.T)                               # [16, O]
    bb = np.asarray(b_base, dtype=np.float32).reshape(1, O)
    brd = np.array([[np.float32(b_router[1]) - np.float32(b_router[0])]], dtype=np.float32)

    in_maps = []
    for c in range(NCORES):
        in_maps.append({
            "xs": x_flat[c * TOK:(c + 1) * TOK],
            "wt": wt,
            "aat": aat,
            "bt": bt,
            "bb": bb,
            "brd": brd,
        })
    return in_maps


def kernel(x, W_base, b_base, A, B, W_router, b_router):
    from concourse import bass_utils

    if "nc" not in _CACHE:
        _CACHE["nc"] = _build()
    nc = _CACHE["nc"]

    in_maps = _prep_host(x, W_base, b_base, A, B, W_router, b_router)
    res = bass_utils.run_bass_kernel_spmd(nc, in_maps, core_ids=list(range(NCORES)))
    out = np.concatenate([res.results[c]["out"] for c in range(NCORES)], axis=0)
    return out.reshape(np.asarray(x).shape[0], -1, O)


# revision 6
# speedup vs baseline: 1.0458x; 1.0458x over previous
"""Trainium2 Bass kernel for the BEMv13 MoE-LoRA module.

Computation (per token t, full problem):
  base  = x @ W_base.T + b_base
  w     = softmax(x @ W_router + b_router)        # E=2 experts
  H     = x @ A_cat.T                             # [T, 16] LoRA down-proj, both experts
  G     = H * w_broadcast * (alpha/rank)          # per-expert routing weight
  out   = base + G @ B_cat.T

Sharding: tokens (batch*seq = 16384) split evenly across 8 NeuronCores;
all weights replicated. No cross-core communication.

On-core algorithm (per core, 2048 tokens):
  - W_base is pre-transposed on host to W^T [D, O]; rounded on-chip to
    float32r (TF32-like) and kept resident in SBUF (128 KB/partition).
  - x arrives token-major; each [128,128] tile is transposed on the PE
    (fp32 transpose) and rounded to float32r during the PSUM->SBUF drain.
  - Main matmul: out[128 tok, 512 o] accumulated over 16 k-tiles in PSUM,
    float32r operands (1 cycle/row, ~fp32 dynamic range, ~1.3e-4 rel err).
  - Router logits difference and LoRA H are fused into one small rhs
    (aat, [D, 18]) sharing the same stationary x^T tiles.
  - softmax over 2 experts == sigmoid of the logit difference.
  - G^T (PE transpose of the scaled H) feeds a final K=16 accumulation
    step with B_cat^T, so the LoRA up-proj lands in the same PSUM banks.
"""

import numpy as np

P = 128
D = 2048
O = 2048
KT = D // P            # 16 k-tiles
TOK = 2048             # tokens per core
NSTR = TOK // 256      # 8 stripes of 256 tokens (2 tok-tiles)
HN = 18                # 16 LoRA cols + 1 router-diff col + 1 pad (fp32r needs even N)
ER = 16                # E*R
SCALE = 16.0 / 8.0
NCORES = 8

_CACHE = {}


def _build():
    import concourse.tile as tile
    import concourse.masks as masks
    from concourse import bacc, mybir

    f32 = mybir.dt.float32
    f32r = mybir.dt.float32r

    nc = bacc.Bacc("TRN2", target_bir_lowering=False, debug=False)

    xs_d = nc.dram_tensor("xs", [TOK, D], f32, kind="ExternalInput")
    wt_d = nc.dram_tensor("wt", [D, O], f32, kind="ExternalInput")
    aat_d = nc.dram_tensor("aat", [P, KT * HN], f32, kind="ExternalInput")
    bt_d = nc.dram_tensor("bt", [ER, O], f32, kind="ExternalInput")
    bb_d = nc.dram_tensor("bb", [1, O], f32, kind="ExternalInput")
    brd_d = nc.dram_tensor("brd", [1, 1], f32, kind="ExternalInput")
    out_d = nc.dram_tensor("out", [TOK, O], f32, kind="ExternalOutput")

    with tile.TileContext(nc) as tc:
        with (
            tc.tile_pool(name="res", bufs=1) as res,
            tc.tile_pool(name="big2k", bufs=2) as big2k,
            tc.tile_pool(name="xpool", bufs=1) as xpool,
            tc.tile_pool(name="xtpool", bufs=18) as xtpool,
            tc.tile_pool(name="small", bufs=2) as small,
            tc.tile_pool(name="psA", bufs=1, space="PSUM") as psA,
            tc.tile_pool(name="psT", bufs=2, space="PSUM") as psT,
            tc.tile_pool(name="psH", bufs=2, space="PSUM") as psH,
        ):
            ident = res.tile([P, P], f32, tag="ident")
            masks.make_identity(nc, ident[:])
            identr = res.tile([P, P], f32r, tag="identr")
            nc.vector.tensor_copy(identr[:], ident[:])

            # stripe-0 x load first: SWDGE path, independent of the HWDGE
            # queue that streams W^T — PE transposes can start immediately.
            x32_tiles = [None] * NSTR
            x32_tiles[0] = xpool.tile([P, 2 * D], f32, tag="x32", name="x32_0")
            nc.gpsimd.dma_start(
                x32_tiles[0][:].rearrange("p (i d) -> p i d", d=D),
                xs_d[0:256, :].rearrange("(i p) d -> p i d", p=P),
            )

            # --- constants: biases (replicated across partitions), A_aug^T, B_cat^T
            bb128 = res.tile([P, O], f32, tag="bb128")
            bb1 = big2k.tile([P, O], f32, tag="big2k", name="bb1_stage")
            nc.sync.dma_start(bb1[:1, :], bb_d[:])
            nc.gpsimd.partition_broadcast(bb128[:], bb1[:1, :])

            brd128 = res.tile([P, 1], f32, tag="brd128")
            brd1 = res.tile([1, 1], f32, tag="brd1")
            nc.sync.dma_start(brd1[:], brd_d[:])
            nc.gpsimd.partition_broadcast(brd128[:], brd1[:])

            aat32 = res.tile([P, KT * HN], f32, tag="aat32")
            nc.sync.dma_start(aat32[:], aat_d[:])
            aat_r = res.tile([P, KT * HN], f32r, tag="aat_r")
            nc.vector.tensor_copy(aat_r[:], aat32[:])

            bt_r = res.tile([ER, O], f32r, tag="bt_r")
            bt32 = big2k.tile([P, O], f32, tag="big2k", name="bt32_stage")
            nc.sync.dma_start(bt32[:ER, :], bt_d[:])
            nc.vector.tensor_copy(bt_r[:], bt32[:ER, :])

            # --- W^T resident in SBUF as float32r: [d-part, k-tile * O]
            wt_r = res.tile([P, KT * O], f32r, tag="wt_r")
            for k in range(KT):
                w32 = big2k.tile([P, O], f32, tag="big2k", name=f"w32_{k}")
                nc.sync.dma_start(w32[:], wt_d[k * P:(k + 1) * P, :])
                nc.vector.tensor_copy(wt_r[:, k * O:(k + 1) * O], w32[:])

            # --- main stripe loop: 256 tokens (2 tok-tiles) per stripe
            for s in range(NSTR):
                if x32_tiles[s] is None:
                    x32_tiles[s] = xpool.tile([P, 2 * D], f32, tag="x32", name=f"x32_{s}")
                    nc.gpsimd.dma_start(
                        x32_tiles[s][:].rearrange("p (i d) -> p i d", d=D),
                        xs_d[s * 256:(s + 1) * 256, :].rearrange("(i p) d -> p i d", p=P),
                    )
                x32 = x32_tiles[s]

                # transpose x tiles: [128 tok, 128 d] -> [128 d, 256 tok] per k
                xts = []
                for k in range(KT):
                    xt = xtpool.tile([P, 256], f32r, tag="xt", name=f"xt_{s}_{k}")
                    xts.append(xt)
                for k2 in range(0, KT, 2):
                    st = psT.tile([P, 512], f32, tag="tstage", name=f"ts_{s}_{k2}")
                    for q in range(2):
                        for i in range(2):
                            nc.tensor.transpose(
                                st[:, q * 256 + i * P:q * 256 + (i + 1) * P],
                                x32[:, i * D + (k2 + q) * P:i * D + (k2 + q + 1) * P],
                                ident[:],
                            )
                    nc.vector.tensor_copy(xts[k2][:], st[:, 0:256])
                    nc.vector.tensor_copy(xts[k2 + 1][:], st[:, 256:512])

                for i in range(2):
                    accs = [
                        psA.tile([P, 512], f32, tag=f"acc{j}", name=f"acc_{s}_{i}_{j}")
                        for j in range(4)
                    ]
                    h = psH.tile([P, HN], f32, tag="h", name=f"h_{s}_{i}")
                    for k in range(KT):
                        lhs = xts[k][:, i * P:(i + 1) * P]
                        nc.tensor.matmul(h[:], lhs, aat_r[:, k * HN:(k + 1) * HN],
                                         start=(k == 0), stop=(k == KT - 1))
                        for j in range(4):
                            nc.tensor.matmul(
                                accs[j][:], lhs,
                                wt_r[:, k * O + j * 512:k * O + (j + 1) * 512],
                                start=(k == 0), stop=False,
                            )

                    # routing: w1 = sigmoid(dlogit + brd); scaled by alpha/rank
                    srow = small.tile([P, 1], f32, tag="srow", name=f"srow_{s}_{i}")
                    nc.scalar.activation(srow[:], h[:, ER:ER + 1],
                                         mybir.ActivationFunctionType.Sigmoid,
                                         bias=brd128[:, 0:1], scale=1.0)
                    w1s = small.tile([P, 1], f32, tag="w1s", name=f"w1s_{s}_{i}")
                    nc.vector.tensor_scalar_mul(w1s[:], srow[:], SCALE)
                    w0s = small.tile([P, 1], f32, tag="w0s", name=f"w0s_{s}_{i}")
                    nc.vector.tensor_scalar(w0s[:], srow[:], -SCALE, SCALE,
                                            mybir.AluOpType.mult, mybir.AluOpType.add)
                    g = small.tile([P, ER], f32r, tag="g", name=f"g_{s}_{i}")
                    nc.vector.tensor_scalar_mul(g[:, 0:8], h[:, 0:8], w0s[:])
                    nc.vector.tensor_scalar_mul(g[:, 8:16], h[:, 8:16], w1s[:])

                    gst = psT.tile([ER, P], f32r, tag="tstage", name=f"gst_{s}_{i}")
                    nc.tensor.transpose(gst[:], g[:], identr[:])
                    gt = small.tile([ER, P], f32r, tag="gt", name=f"gt_{s}_{i}")
                    nc.vector.tensor_copy(gt[:], gst[:])

                    for j in range(4):
                        nc.tensor.matmul(accs[j][:], gt[:],
                                         bt_r[:, j * 512:(j + 1) * 512],
                                         start=False, stop=True)

                    outt = big2k.tile([P, O], f32, tag="big2k", name=f"out_{s}_{i}")
                    for j in range(4):
                        nc.vector.tensor_add(outt[:, j * 512:(j + 1) * 512],
                                             accs[j][:], bb128[:, j * 512:(j + 1) * 512])
                    nc.sync.dma_start(
                        out_d[s * 256 + i * P:s * 256 + (i + 1) * P, :], outt[:])

    nc.compile()
    return nc


def _prep_host(x, W_base, b_base, A, B, W_router, b_router):
    """Host-side layout prep + sharding. Returns per-core input maps."""
    x_flat = np.ascontiguousarray(x, dtype=np.float32).reshape(-1, D)
    wt = np.ascontiguousarray(W_base.T, dtype=np.float32)           # [D, O]
    a_cat = np.asarray(A, dtype=np.float32).reshape(ER, D)          # [16, D]
    aat = np.zeros((D, HN), dtype=np.float32)
    aat[:, :ER] = a_cat.T
    aat[:, ER] = np.asarray(W_router, dtype=np.float32)[:, 1] - np.asarray(W_router, dtype=np.float32)[:, 0]
    # pre-arrange for contiguous per-partition DMA: [P, KT*HN]
    aat = np.ascontiguousarray(aat.reshape(KT, P, HN).transpose(1, 0, 2).reshape(P, KT * HN))
    b_cat = np.concatenate([np.asarray(B, dtype=np.float32)[0],
                            np.asarray(B, dtype=np.float32)[1]], axis=1)  # [O, 16]
    bt = np.ascontiguousarray(b_cat.T)                               # [16, O]
    bb = np.asarray(b_base, dtype=np.float32).reshape(1, O)
    brd = np.array([[np.float32(b_router[1]) - np.float32(b_router[0])]], dtype=np.float32)

    in_maps = []
    for c in range(NCORES):
        in_maps.append({
            "xs": x_flat[c * TOK:(c + 1) * TOK],
            "wt": wt,
            "aat": aat,
            "bt": bt,
            "bb": bb,
            "brd": brd,
        })
    return in_maps


def kernel(x, W_base, b_base, A, B, W_router, b_router):
    from concourse import bass_utils

    if "nc" not in _CACHE:
        _CACHE["nc"] = _build()
    nc = _CACHE["nc"]

    in_maps = _prep_host(x, W_base, b_base, A, B, W_router, b_router)
    res = bass_utils.run_bass_kernel_spmd(nc, in_maps, core_ids=list(range(NCORES)))
    out = np.concatenate([res.results[c]["out"] for c in range(NCORES)], axis=0)
    return out.reshape(np.asarray(x).shape[0], -1, O)


# revision 7
# speedup vs baseline: 1.1088x; 1.0602x over previous
"""Trainium2 Bass kernel for the BEMv13 MoE-LoRA module.

Computation (per token t, full problem):
  base  = x @ W_base.T + b_base
  w     = softmax(x @ W_router + b_router)        # E=2 experts
  H     = x @ A_cat.T                             # [T, 16] LoRA down-proj, both experts
  G     = H * w_broadcast * (alpha/rank)          # per-expert routing weight
  out   = base + G @ B_cat.T

Sharding: tokens (batch*seq = 16384) split evenly across 8 NeuronCores;
all weights replicated. No cross-core communication.

On-core algorithm (per core, 2048 tokens):
  - W_base is pre-transposed on host to W^T [D, O]; rounded on-chip to
    float32r (TF32-like) and kept resident in SBUF (128 KB/partition).
  - x arrives token-major; each [128,128] tile is transposed on the PE
    (fp32 transpose) and rounded to float32r during the PSUM->SBUF drain.
  - Main matmul: out[128 tok, 512 o] accumulated over 16 k-tiles in PSUM,
    float32r operands (1 cycle/row, ~fp32 dynamic range, ~1.3e-4 rel err).
  - Router logits difference and LoRA H are fused into one small rhs
    (aat, [D, 18]) sharing the same stationary x^T tiles.
  - softmax over 2 experts == sigmoid of the logit difference.
  - G^T (PE transpose of the scaled H) feeds a final K=16 accumulation
    step with B_cat^T, so the LoRA up-proj lands in the same PSUM banks.
"""

import numpy as np

P = 128
D = 2048
O = 2048
KT = D // P            # 16 k-tiles
TOK = 2048             # tokens per core
NSTR = TOK // 256      # 8 stripes of 256 tokens (2 tok-tiles)
HN = 18                # 16 LoRA cols + 1 router-diff col + 1 pad (fp32r needs even N)
ER = 16                # E*R
SCALE = 16.0 / 8.0
NCORES = 8

_CACHE = {}


def _build():
    import concourse.tile as tile
    import concourse.masks as masks
    from concourse import bacc, mybir

    f32 = mybir.dt.float32
    f32r = mybir.dt.float32r

    nc = bacc.Bacc("TRN2", target_bir_lowering=False, debug=False)

    xs_d = nc.dram_tensor("xs", [TOK, D], f32, kind="ExternalInput")
    wt_d = nc.dram_tensor("wt", [D, O], f32, kind="ExternalInput")
    aat_d = nc.dram_tensor("aat", [P, KT * HN], f32, kind="ExternalInput")
    bt_d = nc.dram_tensor("bt", [ER, O], f32, kind="ExternalInput")
    bb_d = nc.dram_tensor("bb", [1, O], f32, kind="ExternalInput")
    brd_d = nc.dram_tensor("brd", [1, 1], f32, kind="ExternalInput")
    out_d = nc.dram_tensor("out", [TOK, O], f32, kind="ExternalOutput")

    with tile.TileContext(nc) as tc:
        with (
            tc.tile_pool(name="res", bufs=1) as res,
            tc.tile_pool(name="big2k", bufs=2) as big2k,
            tc.tile_pool(name="xpool", bufs=1) as xpool,
            tc.tile_pool(name="xtpool", bufs=18) as xtpool,
            tc.tile_pool(name="small", bufs=2) as small,
            tc.tile_pool(name="psA", bufs=5, space="PSUM") as psA,
            tc.tile_pool(name="psT", bufs=2, space="PSUM") as psT,
            tc.tile_pool(name="psH", bufs=1, space="PSUM") as psH,
        ):
            ident = res.tile([P, P], f32, tag="ident")
            masks.make_identity(nc, ident[:])
            identr = res.tile([P, P], f32r, tag="identr")
            nc.vector.tensor_copy(identr[:], ident[:])

            # stripe-0 x load first: SWDGE path, independent of the HWDGE
            # queue that streams W^T — PE transposes can start immediately.
            x32_tiles = [None] * NSTR
            x32_tiles[0] = xpool.tile([P, 2 * D], f32, tag="x32", name="x32_0")
            for i in range(2):
                for hh in range(2):
                    nc.gpsimd.dma_start(
                        x32_tiles[0][:, i * D + hh * (D // 2):i * D + (hh + 1) * (D // 2)],
                        xs_d[i * P:(i + 1) * P, hh * (D // 2):(hh + 1) * (D // 2)],
                    )

            # --- constants: biases (replicated across partitions), A_aug^T, B_cat^T
            bb128 = res.tile([P, O], f32, tag="bb128")
            bb1 = big2k.tile([P, O], f32, tag="big2k", name="bb1_stage")
            nc.sync.dma_start(bb1[:1, :], bb_d[:])
            nc.gpsimd.partition_broadcast(bb128[:], bb1[:1, :])

            brd128 = res.tile([P, 1], f32, tag="brd128")
            brd1 = res.tile([1, 1], f32, tag="brd1")
            nc.sync.dma_start(brd1[:], brd_d[:])
            nc.gpsimd.partition_broadcast(brd128[:], brd1[:])

            aat32 = res.tile([P, KT * HN], f32, tag="aat32")
            nc.sync.dma_start(aat32[:], aat_d[:])
            aat_r = res.tile([P, KT * HN], f32r, tag="aat_r")
            nc.vector.tensor_copy(aat_r[:], aat32[:])

            bt_r = res.tile([ER, O], f32r, tag="bt_r")
            bt32 = big2k.tile([P, O], f32, tag="big2k", name="bt32_stage")
            nc.sync.dma_start(bt32[:ER, :], bt_d[:])
            nc.vector.tensor_copy(bt_r[:], bt32[:ER, :])

            # --- W^T resident in SBUF as float32r: [d-part, k-tile * O]
            wt_r = res.tile([P, KT * O], f32r, tag="wt_r")
            for k in range(KT):
                w32 = big2k.tile([P, O], f32, tag="big2k", name=f"w32_{k}")
                nc.sync.dma_start(w32[:], wt_d[k * P:(k + 1) * P, :])
                nc.vector.tensor_copy(wt_r[:, k * O:(k + 1) * O], w32[:])

            # --- main stripe loop: 256 tokens (2 tok-tiles) per stripe
            for s in range(NSTR):
                if x32_tiles[s] is None:
                    x32_tiles[s] = xpool.tile([P, 2 * D], f32, tag="x32", name=f"x32_{s}")
                    for i in range(2):
                        nc.gpsimd.dma_start(
                            x32_tiles[s][:, i * D:(i + 1) * D],
                            xs_d[s * 256 + i * P:s * 256 + (i + 1) * P, :],
                        )
                x32 = x32_tiles[s]

                # transpose x tiles: [128 tok, 128 d] -> [128 d, 256 tok] per k
                xts = []
                for k in range(KT):
                    xt = xtpool.tile([P, 256], f32r, tag="xt", name=f"xt_{s}_{k}")
                    xts.append(xt)
                for k2 in range(0, KT, 2):
                    st = psT.tile([P, 512], f32, tag="tstage", name=f"ts_{s}_{k2}")
                    for q in range(2):
                        for i in range(2):
                            nc.tensor.transpose(
                                st[:, q * 256 + i * P:q * 256 + (i + 1) * P],
                                x32[:, i * D + (k2 + q) * P:i * D + (k2 + q + 1) * P],
                                ident[:],
                            )
                    nc.scalar.copy(xts[k2][:], st[:, 0:256])
                    nc.scalar.copy(xts[k2 + 1][:], st[:, 256:512])

                for i in range(2):
                    accs = [
                        psA.tile([P, 512], f32, tag="acc", name=f"acc_{s}_{i}_{j}")
                        for j in range(4)
                    ]
                    h = psH.tile([P, HN], f32, tag="h", name=f"h_{s}_{i}")
                    for k in range(KT):
                        lhs = xts[k][:, i * P:(i + 1) * P]
                        nc.tensor.matmul(h[:], lhs, aat_r[:, k * HN:(k + 1) * HN],
                                         start=(k == 0), stop=(k == KT - 1))
                        for j in range(4):
                            nc.tensor.matmul(
                                accs[j][:], lhs,
                                wt_r[:, k * O + j * 512:k * O + (j + 1) * 512],
                                start=(k == 0), stop=False,
                            )

                    # routing: w1 = sigmoid(dlogit + brd); scaled by alpha/rank
                    srow = small.tile([P, 1], f32, tag="srow", name=f"srow_{s}_{i}")
                    nc.scalar.activation(srow[:], h[:, ER:ER + 1],
                                         mybir.ActivationFunctionType.Sigmoid,
                                         bias=brd128[:, 0:1], scale=1.0)
                    w1s = small.tile([P, 1], f32, tag="w1s", name=f"w1s_{s}_{i}")
                    nc.vector.tensor_scalar_mul(w1s[:], srow[:], SCALE)
                    w0s = small.tile([P, 1], f32, tag="w0s", name=f"w0s_{s}_{i}")
                    nc.vector.tensor_scalar(w0s[:], srow[:], -SCALE, SCALE,
                                            mybir.AluOpType.mult, mybir.AluOpType.add)
                    g = small.tile([P, ER], f32r, tag="g", name=f"g_{s}_{i}")
                    nc.vector.tensor_scalar_mul(g[:, 0:8], h[:, 0:8], w0s[:])
                    nc.vector.tensor_scalar_mul(g[:, 8:16], h[:, 8:16], w1s[:])

                    gst = psT.tile([ER, P], f32r, tag="tstage", name=f"gst_{s}_{i}")
                    nc.tensor.transpose(gst[:], g[:], identr[:])
                    gt = small.tile([ER, P], f32r, tag="gt", name=f"gt_{s}_{i}")
                    nc.vector.tensor_copy(gt[:], gst[:])

                    for j in range(4):
                        nc.tensor.matmul(accs[j][:], gt[:],
                                         bt_r[:, j * 512:(j + 1) * 512],
                                         start=False, stop=True)

                    outt = big2k.tile([P, O], f32, tag="big2k", name=f"out_{s}_{i}")
                    for j in range(4):
                        nc.vector.tensor_add(outt[:, j * 512:(j + 1) * 512],
                                             accs[j][:], bb128[:, j * 512:(j + 1) * 512])
                    nc.sync.dma_start(
                        out_d[s * 256 + i * P:s * 256 + (i + 1) * P, :], outt[:])

    nc.compile()
    return nc


def _prep_host(x, W_base, b_base, A, B, W_router, b_router):
    """Host-side layout prep + sharding. Returns per-core input maps."""
    x_flat = np.ascontiguousarray(x, dtype=np.float32).reshape(-1, D)
    wt = np.ascontiguousarray(W_base.T, dtype=np.float32)           # [D, O]
    a_cat = np.asarray(A, dtype=np.float32).reshape(ER, D)          # [16, D]
    aat = np.zeros((D, HN), dtype=np.float32)
    aat[:, :ER] = a_cat.T
    aat[:, ER] = np.asarray(W_router, dtype=np.float32)[:, 1] - np.asarray(W_router, dtype=np.float32)[:, 0]
    # pre-arrange for contiguous per-partition DMA: [P, KT*HN]
    aat = np.ascontiguousarray(aat.reshape(KT, P, HN).transpose(1, 0, 2).reshape(P, KT * HN))
    b_cat = np.concatenate([np.asarray(B, dtype=np.float32)[0],
                            np.asarray(B, dtype=np.float32)[1]], axis=1)  # [O, 16]
    bt = np.ascontiguousarray(b_cat.T)                               # [16, O]
    bb = np.asarray(b_base, dtype=np.float32).reshape(1, O)
    brd = np.array([[np.float32(b_router[1]) - np.float32(b_router[0])]], dtype=np.float32)

    in_maps = []
    for c in range(NCORES):
        in_maps.append({
            "xs": x_flat[c * TOK:(c + 1) * TOK],
            "wt": wt,
            "aat": aat,
            "bt": bt,
            "bb": bb,
            "brd": brd,
        })
    return in_maps


def kernel(x, W_base, b_base, A, B, W_router, b_router):
    from concourse import bass_utils

    if "nc" not in _CACHE:
        _CACHE["nc"] = _build()
    nc = _CACHE["nc"]

    in_maps = _prep_host(x, W_base, b_base, A, B, W_router, b_router)
    res = bass_utils.run_bass_kernel_spmd(nc, in_maps, core_ids=list(range(NCORES)))
    out = np.concatenate([res.results[c]["out"] for c in range(NCORES)], axis=0)
    return out.reshape(np.asarray(x).shape[0], -1, O)
